# revision 23
# baseline (speedup 1.0000x reference)
"""CIF (continuous integrate-and-fire) kernel for Trainium2, 8 NeuronCores.

Strategy
--------
The CIF scan over time only has a *scalar* recurrence: the integrate/fire
decisions and the per-step blend weights depend solely on ``alphas`` [B, T]
(256 KB).  All the heavy work involving ``hidden`` [B, T, H] (131 MB) is,
for fixed fire decisions, a linear map: every output frame j is a weighted
sum of consecutive hidden rows,

    out[b, j, :] = sum_t W[b, t, j] * hidden[b, t, :]

where W[b] is a [T, 64] sparse-banded weight matrix (each time step
contributes to at most two adjacent frames; weights are the reference's
``cur``/``remainds`` values).

So: replicate the reference's fp32 scalar scan on the host (exact same op
order -> bit-identical fire decisions), then run the batched
[64, T] @ [T, H] matmul on the 8 NeuronCores — pure data parallel over the
batch dim, 4 rows per core, K-tiled over T with PSUM accumulation.

W is streamed in COMPACT form (cur/rem/idx/idx+1 per time step, 128 B per
partition per row instead of the 2 KB dense [T, 64] band matrix) and
expanded on the idle vector engine:
    W[p, c, m] = cur[p,c]*(m == idx[p,c]) + rem[p,c]*(m == idx[p,c]+1)
via an iota over m and two is_equal/mult plus one add, all [128, 16, 64]
ops with stride-0 broadcast APs.  This cuts ~1 MB/core off the DMA
stream, which is the binding resource (~24 GB/s x 16 DMA engines).

RAW BASS version (no TileContext), hand-rolled synchronization:
  - one completion semaphore per DMA trigger (the 16 HW DMA engines
    each bump it +1 for their share; >=16 means the transfer landed);
  - rings self-pace on their own trigger completions (trigger #n waits
    for #n-DEPTH), keeping HWDGE queues shallow (deep queues measurably
    drop per-packet HBM rate ~15%) while staying decoupled from PE
    speed (the HAM clock gate makes PE speed erratic);
  - row_done: each row's stop matmul bumps it; releases the row's
    PSUM->SBUF cast on the vector engine;
  - cast_done: gates the output DMA triggers, issued last on the rings
    so they can never head-of-line-block hidden streaming.
All buffers are unique SBUF allocations, so there are no WAR hazards.
"""

import os

import numpy as np

# --- problem constants (hardcoded per spec: nn_CIF_Model_5970004541927) ---
B, T, H = 32, 2000, 512
NCORES = 8
R = B // NCORES          # batch rows per core = 4
ML = 64                  # MAX_LABELS
THRESH = np.float32(0.95)
P = 128                  # SBUF partitions
NFULL = T // P           # 15 full K-chunks
TAIL = T - NFULL * P     # 80 leftover time steps
NCHUNK = NFULL + 1       # 16
TP = NCHUNK * P          # 2048 (weights padded so chunks divide evenly)
GRP = int(os.environ.get("CIF_GRP", "4"))  # K-chunks per hidden DMA
DEPTH = int(os.environ.get("CIF_DEPTH", "4"))  # per-ring in-flight DMA window
# Device-side W expansion from a compact (cur/rem/idx) stream: saves
# ~1 MB/core of DMA but the stride-0 broadcast tensor_tensor ops run at
# only ~107 G elem/s on DVE (5.5 us/row), delaying the PE start and the
# row casts — measured net-negative (40242 vs 39171 ns).  Default off.
WEXP = os.environ.get("CIF_WEXP", "0") == "1"

# matmul dtype on the PE: "fp16" (default), "fp32r", "fp32", or "bf16"
MM_MODE = os.environ.get("CIF_MM_MODE", "fp16")

_COMPILED = {}


def _scan(alphas: np.ndarray):
    """Replicate the reference fp32 scan on alphas only (vectorized over
    batch).  Returns cur [B, T], rem [B, T], idx [B, T] (int, clipped to
    ML): step t adds cur*h_t to frame idx and rem*h_t to frame idx+1
    (frames >= ML are dropped).

    Per time step t (exactly the reference ops):
        dist_completion = 1 - integrate
        integrate += a_t ; fire = integrate > 0.95
        integrate -= fire
        cur = fire ? dist_completion : a_t   -> frame n   (n = fires so far)
        remainds = a_t - cur                 -> frame n+1  (only at a fire)
    """
    Bv, Tv = alphas.shape
    a = np.ascontiguousarray(alphas, dtype=np.float32)
    integrate = np.zeros(Bv, np.float32)
    nfires = np.zeros(Bv, np.int64)
    curs = np.zeros((Bv, Tv), np.float32)
    rems = np.zeros((Bv, Tv), np.float32)
    idxs = np.zeros((Bv, Tv), np.int64)
    one = np.float32(1.0)
    for t in range(Tv):
        a_t = a[:, t]
        dist_completion = one - integrate
        integrate = integrate + a_t
        fire = integrate > THRESH
        integrate = np.where(fire, integrate - one, integrate)
        cur = np.where(fire, dist_completion, a_t)
        remainds = np.where(fire, a_t - cur, np.float32(0.0))
        curs[:, t] = cur
        rems[:, t] = remainds
        idxs[:, t] = np.minimum(nfires, ML)
        nfires = nfires + fire
    return curs, rems, idxs


def _build_weights_dense(curs, rems, idxs) -> np.ndarray:
    """Dense WF [B, P, NCHUNK, ML] float32 (fallback when WEXP=0)."""
    Bv, Tv = curs.shape
    WT = np.zeros((Bv, TP, ML + 2), np.float32)
    rows = np.arange(Bv)[:, None]
    ts = np.arange(Tv)[None, :]
    WT[rows, ts, idxs] = curs
    WT[rows, ts, np.minimum(idxs + 1, ML + 1)] = rems
    WT = WT[:, :, :ML]
    WF = WT.reshape(Bv, NCHUNK, P, ML).transpose(0, 2, 1, 3)
    return np.ascontiguousarray(WF)


def _build_weights_compact(curs, rems, idxs) -> np.ndarray:
    """Compact wc [B, P, 4*NCHUNK] fp16: per partition p the free dim is
    [cur(16) | rem(16) | idx(16) | idx+1(16)], channel c holding the
    value for time step t = c*P + p.  Steps past T contribute zero."""
    Bv = curs.shape[0]
    wc = np.zeros((Bv, 4, NCHUNK, P), np.float32)
    pad = TP - T
    curp = np.pad(curs, ((0, 0), (0, pad))).reshape(Bv, NCHUNK, P)
    remp = np.pad(rems, ((0, 0), (0, pad))).reshape(Bv, NCHUNK, P)
    idxp = np.pad(idxs, ((0, 0), (0, pad))).reshape(Bv, NCHUNK, P)
    wc[:, 0] = curp
    wc[:, 1] = remp
    wc[:, 2] = idxp
    wc[:, 3] = idxp + 1
    # -> [B, P, 4, NCHUNK]
    wc = wc.transpose(0, 3, 1, 2).reshape(Bv, P, 4 * NCHUNK)
    return np.ascontiguousarray(wc.astype(np.float16))


def _build_nc(mm_mode: str):
    """Emit the raw Bass program (identical on all 8 cores; SPMD over batch)."""
    import concourse.bacc as bacc
    import concourse.mybir as mybir
    from concourse.bass import broadcast_tensor_aps

    f32 = mybir.dt.float32
    f16 = mybir.dt.float16
    in_dt = {
        "fp32": f32,
        "fp32r": mybir.dt.float32r,
        "bf16": mybir.dt.bfloat16,
        "fp16": mybir.dt.float16,
    }[mm_mode]
    out_dt = f32 if mm_mode in ("fp32", "fp32r") else f16
    use_wexp = WEXP and mm_mode == "fp16"

    nc = bacc.Bacc("TRN2", target_bir_lowering=False, debug=False)
    hidp = nc.dram_tensor("hidp", [R, P, NFULL * H], in_dt, kind="ExternalInput")
    hidt = nc.dram_tensor("hidt", [R, TAIL, H], in_dt, kind="ExternalInput")
    if use_wexp:
        wt = nc.dram_tensor("wt", [R, P, 4 * NCHUNK], f16, kind="ExternalInput")
    else:
        wt = nc.dram_tensor("wt", [R, P, NCHUNK * ML], in_dt, kind="ExternalInput")
    out = nc.dram_tensor("out", [R, ML, H], out_dt, kind="ExternalOutput")

    def _mk_groups():
        gs, pos = [], 0
        while pos < NFULL:
            gs.append(list(range(pos, min(pos + GRP, NFULL))))
            pos += GRP
        return gs

    groups = _mk_groups()
    NG = len(groups)

    # unique SBUF buffers: no reuse, no WAR hazards
    w_shape = [P, NCHUNK, ML] if use_wexp else [P, NCHUNK * ML]
    w_sb = [nc.alloc_sbuf_tensor(f"w{r}", w_shape, in_dt) for r in range(R)]

    def w_chunk(r, c):
        return w_sb[r][:, c, :] if use_wexp else w_sb[r][:, c * ML : (c + 1) * ML]

    def w_tail(r):
        if use_wexp:
            return w_sb[r][0:TAIL, NFULL, :]
        return w_sb[r][0:TAIL, NFULL * ML : NCHUNK * ML]
    t_sb = [nc.alloc_sbuf_tensor(f"t{r}", [TAIL, H], in_dt) for r in range(R)]
    h_sb = [
        [
            nc.alloc_sbuf_tensor(f"h{r}_{gi}", [P, len(g) * H], in_dt)
            for gi, g in enumerate(groups)
        ]
        for r in range(R)
    ]
    o_sb = [nc.alloc_sbuf_tensor(f"o{r}", [ML, H], out_dt) for r in range(R)]
    ps = [nc.alloc_psum_tensor(f"ps{r}", [ML, H], f32) for r in range(R)]

    row_done = nc.alloc_semaphore("row_done")
    cast_done = nc.alloc_semaphore("cast_done")

    rings = [nc.sync, nc.scalar]
    ring_hist = [[], []]
    di = 0

    def issue(dst, src):
        """Issue a DMA trigger on the next ring.

        Each trigger gets its own completion semaphore: the 16 HW DMA
        engines each bump it +1 when their share of the transfer lands,
        so sem >= 16 means the whole transfer is in SBUF.  (A shared
        per-ring counter does NOT work: different triggers' engine-share
        bumps interleave, so intermediate counts are unordered.)

        Pacing: a ring issues trigger #n only after its own trigger
        #n-DEPTH has fully landed.  This keeps the HWDGE queues at a
        fixed shallow depth (deep queues measurably drop per-packet HBM
        rate ~15%) while keeping DMA progress completely decoupled from
        PE progress (the HAM clock gate makes PE speed erratic).
        """
        nonlocal di
        ri = di % 2
        di += 1
        hist = ring_hist[ri]
        if len(hist) >= DEPTH:
            rings[ri].wait_ge(hist[-DEPTH], 16)
        sem = nc.alloc_semaphore(f"dma{di}")
        rings[ri].dma_start(dst, src).then_inc(sem, 16)
        hist.append(sem)
        return sem

    # ---- W expansion prep (iota over the frame index m) ----
    if use_wexp:
        wc_sb = [
            nc.alloc_sbuf_tensor(f"wc{r}", [P, 4 * NCHUNK], f16) for r in range(R)
        ]
        iota_f = nc.alloc_sbuf_tensor("iota_f", [P, ML], f16)
        ea = nc.alloc_sbuf_tensor("ea", [P, NCHUNK, ML], f16)
        eb = nc.alloc_sbuf_tensor("eb", [P, NCHUNK, ML], f16)
        iota_done = nc.alloc_semaphore("iota_done")
        wexp_done = nc.alloc_semaphore("wexp_done")
        nc.gpsimd.iota(
            iota_f[:],
            pattern=[[1, ML]],
            base=0,
            channel_multiplier=0,
            allow_small_or_imprecise_dtypes=True,
        ).then_inc(iota_done)

    # ---- input DMA triggers: [w, tail, g0..gN] per row ----
    w_dep, t_dep, h_dep = [], [], []
    for r in range(R):
        if use_wexp:
            w_dep.append(issue(wc_sb[r][:], wt[r]))
        else:
            w_dep.append(issue(w_sb[r][:], wt[r]))
        t_dep.append(issue(t_sb[r][:], hidt[r]))
        deps = []
        for gi, g in enumerate(groups):
            deps.append(
                issue(h_sb[r][gi][:], hidp[r][:, g[0] * H : (g[-1] + 1) * H])
            )
        h_dep.append(deps)

    # ---- W expansion on the vector engine (rows in order) ----
    if use_wexp:
        is_eq = None
        import concourse.mybir as _mybir

        op_eq = _mybir.AluOpType.is_equal
        op_mul = _mybir.AluOpType.mult
        op_add = _mybir.AluOpType.add
        nc.vector.wait_ge(iota_done, 1)
        for r in range(R):
            nc.vector.wait_ge(w_dep[r], 16)
            wc = wc_sb[r]
            cur_c = wc[:, 0 * NCHUNK : 1 * NCHUNK][:, :, None]
            rem_c = wc[:, 1 * NCHUNK : 2 * NCHUNK][:, :, None]
            idx_c = wc[:, 2 * NCHUNK : 3 * NCHUNK][:, :, None]
            idx1_c = wc[:, 3 * NCHUNK : 4 * NCHUNK][:, :, None]
            iota_b = iota_f[:, None, :]

            def tt(out_ap, a_ap, b_ap, op):
                a2, b2 = broadcast_tensor_aps(a_ap, b_ap)
                return nc.vector.tensor_tensor(out_ap, a2, b2, op)

            tt(ea[:], iota_b, idx_c, op_eq)        # ea = (m == idx)
            tt(ea[:], ea[:], cur_c, op_mul)        # ea *= cur
            tt(eb[:], iota_b, idx1_c, op_eq)       # eb = (m == idx+1)
            tt(eb[:], eb[:], rem_c, op_mul)        # eb *= rem
            nc.vector.tensor_tensor(w_sb[r][:], ea[:], eb[:], op_add).then_inc(
                wexp_done
            )

    # ---- matmul chains (PE in order); each row's stop matmul bumps
    # row_done, which releases the row's cast ----
    pe = nc.tensor
    pe_seen = set()

    def pe_wait(sem, val=16):
        if sem not in pe_seen:
            pe.wait_ge(sem, val)
            pe_seen.add(sem)

    for r in range(R):
        if use_wexp:
            pe.wait_ge(wexp_done, r + 1)
        else:
            pe_wait(w_dep[r])
        for gi, g in enumerate(groups):
            pe_wait(h_dep[r][gi])
            for ci, c in enumerate(g):
                stop = gi == NG - 1 and ci == len(g) - 1
                m = pe.matmul(
                    ps[r][:],
                    w_chunk(r, c),
                    h_sb[r][gi][:, ci * H : (ci + 1) * H],
                    start=(c == 0),
                    stop=stop,
                )
                if stop:
                    m.then_inc(row_done)
            if gi == 0:
                pe_wait(t_dep[r])
                pe.matmul(
                    ps[r][:],
                    w_tail(r),
                    t_sb[r][:],
                    start=False,
                    stop=False,
                )

    # ---- PSUM -> SBUF casts (vector engine, after the expansions) ----
    for r in range(R):
        nc.vector.wait_ge(row_done, r + 1)
        nc.vector.tensor_copy(o_sb[r][:], ps[r][:]).then_inc(cast_done)

    # ---- output triggers LAST on the rings ----
    out_sems = []
    for r in range(R):
        ri = di % 2
        di += 1
        rings[ri].wait_ge(cast_done, r + 1)
        sem = nc.alloc_semaphore(f"out{r}")
        rings[ri].dma_start(out[r], o_sb[r][:]).then_inc(sem, 16)
        out_sems.append((ri, sem))

    # make sure the program does not end before the output DMAs land
    for ri, sem in out_sems:
        rings[ri].wait_ge(sem, 16)

    nc.compile()
    return nc


def _get_nc(mm_mode: str):
    if mm_mode not in _COMPILED:
        _COMPILED[mm_mode] = _build_nc(mm_mode)
    return _COMPILED[mm_mode]


def kernel(hidden: np.ndarray, alphas: np.ndarray, _trace: bool = False):
    from concourse.bass_utils import run_bass_kernel_spmd

    hidden = np.asarray(hidden, dtype=np.float32)
    alphas = np.asarray(alphas, dtype=np.float32)
    assert hidden.shape == (B, T, H) and alphas.shape == (B, T)

    curs, rems, idxs = _scan(alphas)
    use_wexp = WEXP and MM_MODE == "fp16"
    if use_wexp:
        wt_dev = _build_weights_compact(curs, rems, idxs)  # [B, P, 4*NCHUNK] f16
    else:
        WF = _build_weights_dense(curs, rems, idxs)  # [B, P, NCHUNK, ML] f32

    # partition-major repack of the first NFULL*P steps:
    # hidp[b, p, c, h] = hidden[b, c*P + p, h]
    hidp = np.ascontiguousarray(
        hidden[:, : NFULL * P].reshape(B, NFULL, P, H).transpose(0, 2, 1, 3)
    )
    hidt = np.ascontiguousarray(hidden[:, NFULL * P :])

    if MM_MODE == "bf16":
        import ml_dtypes

        hidp = hidp.astype(ml_dtypes.bfloat16)
        hidt = hidt.astype(ml_dtypes.bfloat16)
        WF = WF.astype(ml_dtypes.bfloat16)
    elif MM_MODE == "fp16":
        hidp = hidp.astype(np.float16)
        hidt = hidt.astype(np.float16)

    if not use_wexp:
        if MM_MODE == "fp16":
            WF = WF.astype(np.float16)
        wt_dev = WF.reshape(B, P, NCHUNK * ML)

    hidp = hidp.reshape(B, P, NFULL * H)

    nc = _get_nc(MM_MODE)
    in_maps = [
        {
            "hidp": hidp[c * R : (c + 1) * R],
            "hidt": hidt[c * R : (c + 1) * R],
            "wt": wt_dev[c * R : (c + 1) * R],
        }
        for c in range(NCORES)
    ]
    res = run_bass_kernel_spmd(nc, in_maps, list(range(NCORES)), trace=_trace)
    out = np.concatenate([res.results[c]["out"] for c in range(NCORES)], axis=0)
    out = np.ascontiguousarray(out.astype(np.float32))
    if _trace:
        return out, res
    return out


# revision 25
# speedup vs baseline: 1.0413x; 1.0413x over previous
"""CIF (continuous integrate-and-fire) kernel for Trainium2, 8 NeuronCores.

Strategy
--------
The CIF scan over time only has a *scalar* recurrence: the integrate/fire
decisions and the per-step blend weights depend solely on ``alphas`` [B, T]
(256 KB).  All the heavy work involving ``hidden`` [B, T, H] (131 MB) is,
for fixed fire decisions, a linear map: every output frame j is a weighted
sum of consecutive hidden rows,

    out[b, j, :] = sum_t W[b, t, j] * hidden[b, t, :]

where W[b] is a [T, 64] sparse-banded weight matrix (each time step
contributes to at most two adjacent frames; weights are the reference's
``cur``/``remainds`` values).

So: replicate the reference's fp32 scalar scan on the host (exact same op
order -> bit-identical fire decisions), then run the batched
[64, T] @ [T, H] matmul on the 8 NeuronCores — pure data parallel over the
batch dim, 4 rows per core, K-tiled over T with PSUM accumulation.

W is streamed in COMPACT form (cur/rem/idx/idx+1 per time step, 128 B per
partition per row instead of the 2 KB dense [T, 64] band matrix) and
expanded on the idle vector engine:
    W[p, c, m] = cur[p,c]*(m == idx[p,c]) + rem[p,c]*(m == idx[p,c]+1)
via an iota over m and two is_equal/mult plus one add, all [128, 16, 64]
ops with stride-0 broadcast APs.  This cuts ~1 MB/core off the DMA
stream, which is the binding resource (~24 GB/s x 16 DMA engines).

RAW BASS version (no TileContext), hand-rolled synchronization:
  - one completion semaphore per DMA trigger (the 16 HW DMA engines
    each bump it +1 for their share; >=16 means the transfer landed);
  - rings self-pace on their own trigger completions (trigger #n waits
    for #n-DEPTH), keeping HWDGE queues shallow (deep queues measurably
    drop per-packet HBM rate ~15%) while staying decoupled from PE
    speed (the HAM clock gate makes PE speed erratic);
  - row_done: each row's stop matmul bumps it; releases the row's
    PSUM->SBUF cast on the vector engine;
  - cast_done: gates the output DMA triggers, issued last on the rings
    so they can never head-of-line-block hidden streaming.
All buffers are unique SBUF allocations, so there are no WAR hazards.
"""

import os

import numpy as np

# --- problem constants (hardcoded per spec: nn_CIF_Model_5970004541927) ---
B, T, H = 32, 2000, 512
NCORES = 8
R = B // NCORES          # batch rows per core = 4
ML = 64                  # MAX_LABELS
THRESH = np.float32(0.95)
P = 128                  # SBUF partitions
NFULL = T // P           # 15 full K-chunks
TAIL = T - NFULL * P     # 80 leftover time steps
NCHUNK = NFULL + 1       # 16
TP = NCHUNK * P          # 2048 (weights padded so chunks divide evenly)
GRP = int(os.environ.get("CIF_GRP", "4"))  # K-chunks per hidden DMA
DEPTH = int(os.environ.get("CIF_DEPTH", "4"))  # per-ring in-flight DMA window
# Device-side W expansion from a compact (cur/rem/idx) stream: saves
# ~1 MB/core of DMA but the stride-0 broadcast tensor_tensor ops run at
# only ~107 G elem/s on DVE (5.5 us/row), delaying the PE start and the
# row casts — measured net-negative (40242 vs 39171 ns).  Default off.
WEXP = os.environ.get("CIF_WEXP", "0") == "1"

# matmul dtype on the PE: "fp16" (default), "fp32r", "fp32", or "bf16"
MM_MODE = os.environ.get("CIF_MM_MODE", "fp16")

_COMPILED = {}


def _scan(alphas: np.ndarray):
    """Replicate the reference fp32 scan on alphas only (vectorized over
    batch).  Returns cur [B, T], rem [B, T], idx [B, T] (int, clipped to
    ML): step t adds cur*h_t to frame idx and rem*h_t to frame idx+1
    (frames >= ML are dropped).

    Per time step t (exactly the reference ops):
        dist_completion = 1 - integrate
        integrate += a_t ; fire = integrate > 0.95
        integrate -= fire
        cur = fire ? dist_completion : a_t   -> frame n   (n = fires so far)
        remainds = a_t - cur                 -> frame n+1  (only at a fire)
    """
    Bv, Tv = alphas.shape
    a = np.ascontiguousarray(alphas, dtype=np.float32)
    integrate = np.zeros(Bv, np.float32)
    nfires = np.zeros(Bv, np.int64)
    curs = np.zeros((Bv, Tv), np.float32)
    rems = np.zeros((Bv, Tv), np.float32)
    idxs = np.zeros((Bv, Tv), np.int64)
    one = np.float32(1.0)
    for t in range(Tv):
        a_t = a[:, t]
        dist_completion = one - integrate
        integrate = integrate + a_t
        fire = integrate > THRESH
        integrate = np.where(fire, integrate - one, integrate)
        cur = np.where(fire, dist_completion, a_t)
        remainds = np.where(fire, a_t - cur, np.float32(0.0))
        curs[:, t] = cur
        rems[:, t] = remainds
        idxs[:, t] = np.minimum(nfires, ML)
        nfires = nfires + fire
    return curs, rems, idxs


def _build_weights_dense(curs, rems, idxs) -> np.ndarray:
    """Dense WF [B, P, NCHUNK, ML] float32 (fallback when WEXP=0)."""
    Bv, Tv = curs.shape
    WT = np.zeros((Bv, TP, ML + 2), np.float32)
    rows = np.arange(Bv)[:, None]
    ts = np.arange(Tv)[None, :]
    WT[rows, ts, idxs] = curs
    WT[rows, ts, np.minimum(idxs + 1, ML + 1)] = rems
    WT = WT[:, :, :ML]
    WF = WT.reshape(Bv, NCHUNK, P, ML).transpose(0, 2, 1, 3)
    return np.ascontiguousarray(WF)


def _build_weights_compact(curs, rems, idxs) -> np.ndarray:
    """Compact wc [B, P, 4*NCHUNK] fp16: per partition p the free dim is
    [cur(16) | rem(16) | idx(16) | idx+1(16)], channel c holding the
    value for time step t = c*P + p.  Steps past T contribute zero."""
    Bv = curs.shape[0]
    wc = np.zeros((Bv, 4, NCHUNK, P), np.float32)
    pad = TP - T
    curp = np.pad(curs, ((0, 0), (0, pad))).reshape(Bv, NCHUNK, P)
    remp = np.pad(rems, ((0, 0), (0, pad))).reshape(Bv, NCHUNK, P)
    idxp = np.pad(idxs, ((0, 0), (0, pad))).reshape(Bv, NCHUNK, P)
    wc[:, 0] = curp
    wc[:, 1] = remp
    wc[:, 2] = idxp
    wc[:, 3] = idxp + 1
    # -> [B, P, 4, NCHUNK]
    wc = wc.transpose(0, 3, 1, 2).reshape(Bv, P, 4 * NCHUNK)
    return np.ascontiguousarray(wc.astype(np.float16))


def _build_nc(mm_mode: str):
    """Emit the raw Bass program (identical on all 8 cores; SPMD over batch)."""
    import concourse.bacc as bacc
    import concourse.mybir as mybir
    from concourse.bass import broadcast_tensor_aps

    f32 = mybir.dt.float32
    f16 = mybir.dt.float16
    in_dt = {
        "fp32": f32,
        "fp32r": mybir.dt.float32r,
        "bf16": mybir.dt.bfloat16,
        "fp16": mybir.dt.float16,
    }[mm_mode]
    out_dt = f32 if mm_mode in ("fp32", "fp32r") else f16
    use_wexp = WEXP and mm_mode == "fp16"

    nc = bacc.Bacc("TRN2", target_bir_lowering=False, debug=False)
    hidp = nc.dram_tensor("hidp", [R, P, NFULL * H], in_dt, kind="ExternalInput")
    hidt = nc.dram_tensor("hidt", [R, TAIL, H], in_dt, kind="ExternalInput")
    if use_wexp:
        wt = nc.dram_tensor("wt", [R, P, 4 * NCHUNK], f16, kind="ExternalInput")
    else:
        wt = nc.dram_tensor("wt", [R, P, NCHUNK * ML], in_dt, kind="ExternalInput")
    out = nc.dram_tensor("out", [R, ML, H], out_dt, kind="ExternalOutput")

    def _mk_groups():
        gs, pos = [], 0
        while pos < NFULL:
            gs.append(list(range(pos, min(pos + GRP, NFULL))))
            pos += GRP
        return gs

    groups = _mk_groups()
    NG = len(groups)

    # unique SBUF buffers: no reuse, no WAR hazards
    w_shape = [P, NCHUNK, ML] if use_wexp else [P, NCHUNK * ML]
    w_sb = [nc.alloc_sbuf_tensor(f"w{r}", w_shape, in_dt) for r in range(R)]

    def w_chunk(r, c):
        return w_sb[r][:, c, :] if use_wexp else w_sb[r][:, c * ML : (c + 1) * ML]

    def w_tail(r):
        if use_wexp:
            return w_sb[r][0:TAIL, NFULL, :]
        return w_sb[r][0:TAIL, NFULL * ML : NCHUNK * ML]
    t_sb = [nc.alloc_sbuf_tensor(f"t{r}", [TAIL, H], in_dt) for r in range(R)]
    h_sb = [
        [
            nc.alloc_sbuf_tensor(f"h{r}_{gi}", [P, len(g) * H], in_dt)
            for gi, g in enumerate(groups)
        ]
        for r in range(R)
    ]
    o_sb = [nc.alloc_sbuf_tensor(f"o{r}", [ML, H], out_dt) for r in range(R)]
    ps = [nc.alloc_psum_tensor(f"ps{r}", [ML, H], f32) for r in range(R)]

    row_done = nc.alloc_semaphore("row_done")
    cast_done = nc.alloc_semaphore("cast_done")

    rings = [nc.sync, nc.scalar]
    ring_hist = [[], []]
    di = 0

    def issue(dst, src):
        """Issue a DMA trigger on the next ring.

        Each trigger gets its own completion semaphore: the 16 HW DMA
        engines each bump it +1 when their share of the transfer lands,
        so sem >= 16 means the whole transfer is in SBUF.  (A shared
        per-ring counter does NOT work: different triggers' engine-share
        bumps interleave, so intermediate counts are unordered.)

        Pacing: a ring issues trigger #n only after its own trigger
        #n-DEPTH has fully landed.  This keeps the HWDGE queues at a
        fixed shallow depth (deep queues measurably drop per-packet HBM
        rate ~15%) while keeping DMA progress completely decoupled from
        PE progress (the HAM clock gate makes PE speed erratic).
        """
        nonlocal di
        ri = di % 2
        di += 1
        hist = ring_hist[ri]
        if len(hist) >= DEPTH:
            rings[ri].wait_ge(hist[-DEPTH], 16)
        sem = nc.alloc_semaphore(f"dma{di}")
        rings[ri].dma_start(dst, src).then_inc(sem, 16)
        hist.append(sem)
        return sem

    # ---- W expansion prep (iota over the frame index m) ----
    if use_wexp:
        wc_sb = [
            nc.alloc_sbuf_tensor(f"wc{r}", [P, 4 * NCHUNK], f16) for r in range(R)
        ]
        iota_f = nc.alloc_sbuf_tensor("iota_f", [P, ML], f16)
        ea = nc.alloc_sbuf_tensor("ea", [P, NCHUNK, ML], f16)
        eb = nc.alloc_sbuf_tensor("eb", [P, NCHUNK, ML], f16)
        iota_done = nc.alloc_semaphore("iota_done")
        wexp_done = nc.alloc_semaphore("wexp_done")
        nc.gpsimd.iota(
            iota_f[:],
            pattern=[[1, ML]],
            base=0,
            channel_multiplier=0,
            allow_small_or_imprecise_dtypes=True,
        ).then_inc(iota_done)

    # ---- input DMA triggers: [w, g0, g1, tail, g2, g3] per row.  With
    # the alternating ring assignment this puts w and g0 on OPPOSITE
    # rings so the PE's two first dependencies download in parallel
    # (~1.5 us earlier matmul start); per-ring bytes stay balanced ----
    w_dep, t_dep, h_dep = [], [], []
    for r in range(R):
        if use_wexp:
            w_dep.append(issue(wc_sb[r][:], wt[r]))
        else:
            w_dep.append(issue(w_sb[r][:], wt[r]))
        deps = []
        for gi, g in enumerate(groups):
            deps.append(
                issue(h_sb[r][gi][:], hidp[r][:, g[0] * H : (g[-1] + 1) * H])
            )
            if gi == 1:
                t_dep.append(issue(t_sb[r][:], hidt[r]))
        h_dep.append(deps)

    # ---- W expansion on the vector engine (rows in order) ----
    if use_wexp:
        is_eq = None
        import concourse.mybir as _mybir

        op_eq = _mybir.AluOpType.is_equal
        op_mul = _mybir.AluOpType.mult
        op_add = _mybir.AluOpType.add
        nc.vector.wait_ge(iota_done, 1)
        for r in range(R):
            nc.vector.wait_ge(w_dep[r], 16)
            wc = wc_sb[r]
            cur_c = wc[:, 0 * NCHUNK : 1 * NCHUNK][:, :, None]
            rem_c = wc[:, 1 * NCHUNK : 2 * NCHUNK][:, :, None]
            idx_c = wc[:, 2 * NCHUNK : 3 * NCHUNK][:, :, None]
            idx1_c = wc[:, 3 * NCHUNK : 4 * NCHUNK][:, :, None]
            iota_b = iota_f[:, None, :]

            def tt(out_ap, a_ap, b_ap, op):
                a2, b2 = broadcast_tensor_aps(a_ap, b_ap)
                return nc.vector.tensor_tensor(out_ap, a2, b2, op)

            tt(ea[:], iota_b, idx_c, op_eq)        # ea = (m == idx)
            tt(ea[:], ea[:], cur_c, op_mul)        # ea *= cur
            tt(eb[:], iota_b, idx1_c, op_eq)       # eb = (m == idx+1)
            tt(eb[:], eb[:], rem_c, op_mul)        # eb *= rem
            nc.vector.tensor_tensor(w_sb[r][:], ea[:], eb[:], op_add).then_inc(
                wexp_done
            )

    # ---- matmul chains (PE in order); each row's stop matmul bumps
    # row_done, which releases the row's cast ----
    pe = nc.tensor
    pe_seen = set()

    def pe_wait(sem, val=16):
        if sem not in pe_seen:
            pe.wait_ge(sem, val)
            pe_seen.add(sem)

    for r in range(R):
        if use_wexp:
            pe.wait_ge(wexp_done, r + 1)
        else:
            pe_wait(w_dep[r])
        for gi, g in enumerate(groups):
            pe_wait(h_dep[r][gi])
            for ci, c in enumerate(g):
                stop = gi == NG - 1 and ci == len(g) - 1
                m = pe.matmul(
                    ps[r][:],
                    w_chunk(r, c),
                    h_sb[r][gi][:, ci * H : (ci + 1) * H],
                    start=(c == 0),
                    stop=stop,
                )
                if stop:
                    m.then_inc(row_done)
            if gi == 1:
                # tail matmul after g1: its DMA is issued between g1 and
                # g2, so the data always lands before the PE gets here
                pe_wait(t_dep[r])
                pe.matmul(
                    ps[r][:],
                    w_tail(r),
                    t_sb[r][:],
                    start=False,
                    stop=False,
                )

    # ---- PSUM -> SBUF casts (vector engine, after the expansions) ----
    for r in range(R):
        nc.vector.wait_ge(row_done, r + 1)
        nc.vector.tensor_copy(o_sb[r][:], ps[r][:]).then_inc(cast_done)

    # ---- output triggers LAST on the rings ----
    out_sems = []
    for r in range(R):
        ri = di % 2
        di += 1
        rings[ri].wait_ge(cast_done, r + 1)
        sem = nc.alloc_semaphore(f"out{r}")
        rings[ri].dma_start(out[r], o_sb[r][:]).then_inc(sem, 16)
        out_sems.append((ri, sem))

    # make sure the program does not end before the output DMAs land
    for ri, sem in out_sems:
        rings[ri].wait_ge(sem, 16)

    nc.compile()
    return nc


def _get_nc(mm_mode: str):
    if mm_mode not in _COMPILED:
        _COMPILED[mm_mode] = _build_nc(mm_mode)
    return _COMPILED[mm_mode]


def kernel(hidden: np.ndarray, alphas: np.ndarray, _trace: bool = False):
    from concourse.bass_utils import run_bass_kernel_spmd

    hidden = np.asarray(hidden, dtype=np.float32)
    alphas = np.asarray(alphas, dtype=np.float32)
    assert hidden.shape == (B, T, H) and alphas.shape == (B, T)

    curs, rems, idxs = _scan(alphas)
    use_wexp = WEXP and MM_MODE == "fp16"
    if use_wexp:
        wt_dev = _build_weights_compact(curs, rems, idxs)  # [B, P, 4*NCHUNK] f16
    else:
        WF = _build_weights_dense(curs, rems, idxs)  # [B, P, NCHUNK, ML] f32

    # partition-major repack of the first NFULL*P steps:
    # hidp[b, p, c, h] = hidden[b, c*P + p, h]
    hidp = np.ascontiguousarray(
        hidden[:, : NFULL * P].reshape(B, NFULL, P, H).transpose(0, 2, 1, 3)
    )
    hidt = np.ascontiguousarray(hidden[:, NFULL * P :])

    if MM_MODE == "bf16":
        import ml_dtypes

        hidp = hidp.astype(ml_dtypes.bfloat16)
        hidt = hidt.astype(ml_dtypes.bfloat16)
        WF = WF.astype(ml_dtypes.bfloat16)
    elif MM_MODE == "fp16":
        hidp = hidp.astype(np.float16)
        hidt = hidt.astype(np.float16)

    if not use_wexp:
        if MM_MODE == "fp16":
            WF = WF.astype(np.float16)
        wt_dev = WF.reshape(B, P, NCHUNK * ML)

    hidp = hidp.reshape(B, P, NFULL * H)

    nc = _get_nc(MM_MODE)
    in_maps = [
        {
            "hidp": hidp[c * R : (c + 1) * R],
            "hidt": hidt[c * R : (c + 1) * R],
            "wt": wt_dev[c * R : (c + 1) * R],
        }
        for c in range(NCORES)
    ]
    res = run_bass_kernel_spmd(nc, in_maps, list(range(NCORES)), trace=_trace)
    out = np.concatenate([res.results[c]["out"] for c in range(NCORES)], axis=0)
    out = np.ascontiguousarray(out.astype(np.float32))
    if _trace:
        return out, res
    return out
